# revision 2
# baseline (speedup 1.0000x reference)
"""Trainium2 Bass kernel for EpsilonNetGM (forward-diffused GMM score network).

Math (per row x of shape [D]):
    m'_k    = sqrt(acp) * means_k
    logit_k = (x . m'_k)/sigma2 + [log w_k - 0.5*||m'_k||^2/sigma2]
    resp    = softmax_k(logit)
    out     = c * (x - resp @ m'),   c = 1/sqrt(sigma2),  sigma2 = 1 - acp

Data-parallel over 8 NeuronCores: x/out sharded on the batch axis.

Implementation notes (v2 — PE in bf16, fp32 PE matmuls are 4x slower due
to the LOW_HIGH two-pass weight split):
 - Host splits x into bf16 hi/lo (xh + xl ~ x to ~1.6e-5 rel), which the
   device loads TRANSPOSED via the 2-byte DMA xbar (no PE/DVE transposes).
 - mm1: S^T = (M'/s2)^T x^T as three bf16 matmuls (hi*hi + lo*hi + hi*lo)
   accumulated in fp32 PSUM -> logit error ~1e-4 absolute.
 - exp on ScalarE with per-partition bias = logw_adj (k is the partition
   in S^T layout); no max-subtraction (|logits| <= ~60, safe in fp32).
   E is emitted twice (bf16 Eh via one ACT, fp32 E via another), GpSimd
   computes El = E - Eh, giving a bf16 hi/lo pair for E.
 - mm2 uses E^T free-dim slices as stationary weights with an augmented
   moving operand [-M' | 1] so each matmul also produces the softmax
   denominator in an extra PSUM column.
 - Final: out = (V * (c/s)) + fp16(c*x) as one scalar_tensor_tensor per
   128-row block (per-partition scalar = c/s).
"""

import os
import sys

for _p in ("/opt/trn_rl_repo", "/root/.axon_site/_ro/trn_rl_repo"):
    if os.path.isdir(_p) and _p not in sys.path:
        sys.path.insert(0, _p)

import numpy as np
import ml_dtypes
from contextlib import ExitStack

import concourse.bass as bass
import concourse.bacc as bacc
import concourse.tile as tile
from concourse import mybir
from concourse.bass_utils import run_bass_kernel_spmd

N_CORES = 8
N, K, D = 32768, 25, 128
N_PER = N // N_CORES          # 4096 rows per core
SB = 512                      # rows per super-block
NSB = N_PER // SB             # 8 super-blocks per core

F32 = mybir.dt.float32
F16 = mybir.dt.float16
BF16 = mybir.dt.bfloat16
AF = mybir.ActivationFunctionType
OP = mybir.AluOpType


def build_program(c_scale: float):
    nc = bacc.Bacc("TRN2", debug=False)

    xh_d = nc.dram_tensor("xh", [N_PER, D], BF16, kind="ExternalInput").ap()
    xl_d = nc.dram_tensor("xl", [N_PER, D], BF16, kind="ExternalInput").ap()
    xc_d = nc.dram_tensor("xc", [N_PER, D], F16, kind="ExternalInput").ap()
    msh_d = nc.dram_tensor("msh", [D, K], BF16, kind="ExternalInput").ap()
    msl_d = nc.dram_tensor("msl", [D, K], BF16, kind="ExternalInput").ap()
    lw_d = nc.dram_tensor("lw", [K, 1], F32, kind="ExternalInput").ap()
    nmh_d = nc.dram_tensor("nmh", [K, D + 1], BF16, kind="ExternalInput").ap()
    nml_d = nc.dram_tensor("nml", [K, D + 1], BF16, kind="ExternalInput").ap()
    out_d = nc.dram_tensor("out", [N_PER, D], F32, kind="ExternalOutput").ap()

    inv_c = float(1.0 / c_scale)

    with tile.TileContext(nc) as tc, ExitStack() as ctx:
        consts = ctx.enter_context(tc.tile_pool(name="consts", bufs=1))
        xth_p = ctx.enter_context(tc.tile_pool(name="xth", bufs=3))
        xtl_p = ctx.enter_context(tc.tile_pool(name="xtl", bufs=3))
        xc_p = ctx.enter_context(tc.tile_pool(name="xc", bufs=3))
        eth_p = ctx.enter_context(tc.tile_pool(name="eth", bufs=2))
        etf_p = ctx.enter_context(tc.tile_pool(name="etf", bufs=2))
        etl_p = ctx.enter_context(tc.tile_pool(name="etl", bufs=2))
        small_p = ctx.enter_context(tc.tile_pool(name="small", bufs=4))
        out_p = ctx.enter_context(tc.tile_pool(name="outp", bufs=3))
        ps_st = ctx.enter_context(tc.tile_pool(name="ps_st", bufs=2, space="PSUM"))
        ps_v = ctx.enter_context(tc.tile_pool(name="ps_v", bufs=2, space="PSUM"))

        msh = consts.tile([D, K], BF16, name="msh")
        nc.sync.dma_start(msh, msh_d)
        msl = consts.tile([D, K], BF16, name="msl")
        nc.sync.dma_start(msl, msl_d)
        lw = consts.tile([K, 1], F32, name="lw")
        nc.sync.dma_start(lw, lw_d)
        nmh = consts.tile([K, D + 1], BF16, name="nmh")
        nc.sync.dma_start(nmh, nmh_d)
        nml = consts.tile([K, D + 1], BF16, name="nml")
        nc.sync.dma_start(nml, nml_d)

        for s in range(NSB):
            n0 = s * SB

            # x^T hi/lo via 2-byte DMA xbar transpose: xth[d, n'] = xh[n0+n', d]
            xth = xth_p.tile([128, SB], BF16, name="xth")
            nc.sync.dma_start(xth, xh_d[n0:n0 + SB, :], transpose=True)
            xtl = xtl_p.tile([128, SB], BF16, name="xtl")
            nc.sync.dma_start(xtl, xl_d[n0:n0 + SB, :], transpose=True)
            # c*x in fp16, n-major blocks: xc[p, 128b+d] = c*x[n0+128b+p, d]
            xc = xc_p.tile([128, SB], F16, name="xc")
            nc.sync.dma_start(
                xc.rearrange("p (b d) -> p b d", d=D),
                xc_d[n0:n0 + SB, :].rearrange("(b p) d -> p b d", p=128),
            )

            # S^T[k, n'] = x_{n'} . m'_k / sigma2   (3-term bf16 split)
            pst = ps_st.tile([K, SB], F32, name="pst")
            nc.tensor.matmul(pst, lhsT=msh, rhs=xth, start=True, stop=False)
            nc.tensor.matmul(pst, lhsT=msl, rhs=xth, start=False, stop=False)
            nc.tensor.matmul(pst, lhsT=msh, rhs=xtl, start=False, stop=True)

            # E^T = exp(S^T + logw_adj), as a bf16 hi/lo pair
            eth = eth_p.tile([K, SB], BF16, name="eth")
            nc.scalar.activation(eth, pst, AF.Exp, bias=lw[:, 0:1], scale=1.0)
            etf = etf_p.tile([K, SB], F32, name="etf")
            nc.scalar.activation(etf, pst, AF.Exp, bias=lw[:, 0:1], scale=1.0)
            etl = etl_p.tile([K, SB], BF16, name="etl")
            nc.gpsimd.tensor_sub(etl, etf, eth)

            # V_b = E_b @ [-M' | 1]: per 128-row block, E^T slice is the
            # stationary operand; col 128 of the moving operand accumulates
            # the softmax denominator s.
            pv01 = ps_v.tile([128, 2 * (D + 1)], F32, name="pv01")
            pv23 = ps_v.tile([128, 2 * (D + 1)], F32, name="pv23")
            for b in range(4):
                pv = pv01 if b < 2 else pv23
                lo = (b % 2) * (D + 1)
                dst = pv[:, lo:lo + D + 1]
                eh_b = eth[:, 128 * b:128 * (b + 1)]
                el_b = etl[:, 128 * b:128 * (b + 1)]
                nc.tensor.matmul(dst, lhsT=eh_b, rhs=nmh, start=True, stop=False)
                nc.tensor.matmul(dst, lhsT=eh_b, rhs=nml, start=False, stop=False)
                nc.tensor.matmul(dst, lhsT=el_b, rhs=nmh, start=False, stop=True)

            # out_b = V_b * (c/s) + c*x_b
            o4 = out_p.tile([128, SB], F32, name="o4")
            for j, pv in enumerate((pv01, pv23)):
                s_view = pv.rearrange("p (b c) -> p b c", c=D + 1)[:, :, D:D + 1]
                tmp2 = small_p.tile([128, 2], F32, name="tmp2")
                nc.vector.tensor_scalar_mul(tmp2, s_view, inv_c)
                rc2 = small_p.tile([128, 2], F32, name="rc2")
                nc.vector.reciprocal(rc2, tmp2)
                for jj in range(2):
                    b = 2 * j + jj
                    nc.vector.scalar_tensor_tensor(
                        out=o4[:, 128 * b:128 * (b + 1)],
                        in0=pv[:, (D + 1) * jj:(D + 1) * jj + D],
                        scalar=rc2[:, jj:jj + 1],
                        in1=xc[:, 128 * b:128 * (b + 1)],
                        op0=OP.mult,
                        op1=OP.add,
                    )

            nc.sync.dma_start(
                out_d[n0:n0 + SB, :].rearrange("(b p) d -> p b d", p=128),
                o4.rearrange("p (b d) -> p b d", d=D),
            )

    nc.compile()
    return nc


def _host_constants(means, weights, alphas_cumprod, t):
    acp = float(np.asarray(alphas_cumprod, dtype=np.float64)[int(t)])
    sigma2 = 1.0 - acp
    c = 1.0 / np.sqrt(sigma2)
    mprime = np.sqrt(acp) * np.asarray(means, dtype=np.float64)      # [K, D]

    mts = (mprime / sigma2).T.astype(np.float32)                     # [D, K]
    msh = mts.astype(ml_dtypes.bfloat16)
    msl = (mts - msh.astype(np.float32)).astype(ml_dtypes.bfloat16)

    logw = np.log(np.asarray(weights, dtype=np.float64))
    lw = (logw - 0.5 * np.sum(mprime * mprime, axis=1) / sigma2)
    lw = lw.astype(np.float32).reshape(K, 1).copy()

    negm = np.zeros((K, D + 1), dtype=np.float32)
    negm[:, :D] = -mprime.astype(np.float32)
    negm[:, D] = 1.0
    nmh = negm.astype(ml_dtypes.bfloat16)
    nml = (negm - nmh.astype(np.float32)).astype(ml_dtypes.bfloat16)
    nml[:, D] = 0.0  # sums column must accumulate E exactly once

    return float(c), msh, msl, lw, nmh, nml


def _host_split_x(x, c):
    xh = x.astype(ml_dtypes.bfloat16)
    xl = (x - xh.astype(np.float32)).astype(ml_dtypes.bfloat16)
    xc = (np.float32(c) * x).astype(np.float16)
    return xh, xl, xc


def _build(inputs):
    x = np.ascontiguousarray(np.asarray(inputs["x"], dtype=np.float32))
    assert x.shape == (N, D), x.shape
    c, msh, msl, lw, nmh, nml = _host_constants(
        inputs["means"], inputs["weights"], inputs["alphas_cumprod"], inputs["t"]
    )
    xh, xl, xc = _host_split_x(x, c)

    nc = build_program(c)
    in_maps = []
    for i in range(N_CORES):
        sl = slice(i * N_PER, (i + 1) * N_PER)
        in_maps.append({
            "xh": np.ascontiguousarray(xh[sl]),
            "xl": np.ascontiguousarray(xl[sl]),
            "xc": np.ascontiguousarray(xc[sl]),
            "msh": msh, "msl": msl, "lw": lw, "nmh": nmh, "nml": nml,
        })
    return nc, in_maps


def kernel(x, means, weights, alphas_cumprod, t):
    nc, in_maps = _build({
        "x": x, "means": means, "weights": weights,
        "alphas_cumprod": alphas_cumprod, "t": t,
    })
    res = run_bass_kernel_spmd(nc, in_maps, list(range(N_CORES)))
    out = np.concatenate([res.results[i]["out"] for i in range(N_CORES)], axis=0)
    return out.astype(np.float32, copy=False)


if __name__ == "__main__":
    rng = np.random.default_rng(0)
    x = rng.standard_normal((N, D), dtype=np.float32)
    means = 2.0 * rng.standard_normal((K, D)).astype(np.float32)
    w = rng.uniform(0.1, 1.0, K).astype(np.float32)
    weights = w / w.sum()
    betas = np.linspace(1e-4, 0.02, 1000, dtype=np.float32)
    acp = np.cumprod(1.0 - betas).astype(np.float32)
    out = kernel(x, means, weights, acp, 500)
    print("out", out.shape, out.dtype, out[:2, :4])



# revision 6
# speedup vs baseline: 1.9932x; 1.9932x over previous
"""Trainium2 Bass kernel for EpsilonNetGM (forward-diffused GMM score network).

Math (per row x of shape [D]):
    m'_k    = sqrt(acp) * means_k
    logit_k = (x . m'_k)/sigma2 + [log w_k - 0.5*||m'_k||^2/sigma2]
    resp    = softmax_k(logit)
    out     = c * (x - resp @ m'),   c = 1/sqrt(sigma2),  sigma2 = 1 - acp

Data-parallel over 8 NeuronCores: x/out sharded on the batch axis.

v3 — single-precision bf16 pipeline (tolerance is 2e-2; sim rel err 2.3e-3):
 - Host uploads x TWICE: transposed bf16 (for mm1's moving operand — no
   DMA/PE transposes on device) and c*x in f16 natural layout (for the
   final add). The transposed copy's columns are permuted so that
   n = 4p + g within each 512-row superblock: mm2 stationary slices stay
   contiguous AND the xc/out DMAs get 1KB-contiguous 4-row runs.
 - Per 512-row superblock: ONE 512-col mm1 (stationary ms [128,25]),
   one exp ACT (bias = logw_adj per-partition), FOUR 129-col mm2s
   (stationary = E^T 128-col slice, moving = [-m' | 1] so the softmax
   denominator lands in an extra PSUM column), four STTs
   out = V*(c/s) + c*x, f16 output store.
 - mm1 of superblock s+1 is issued before mm2 of superblock s so the PE
   never stalls on the exp latency.
"""

import os
import sys

for _p in ("/opt/trn_rl_repo", "/root/.axon_site/_ro/trn_rl_repo"):
    if os.path.isdir(_p) and _p not in sys.path:
        sys.path.insert(0, _p)

import numpy as np
import ml_dtypes
from contextlib import ExitStack

import concourse.bass as bass
import concourse.bacc as bacc
import concourse.tile as tile
from concourse import mybir
from concourse.bass_utils import run_bass_kernel_spmd

N_CORES = 8
N, K, D = 32768, 25, 128
N_PER = N // N_CORES          # 4096 rows per core
SB = 512                      # rows per super-block
NSB = N_PER // SB             # 8 super-blocks per core

F32 = mybir.dt.float32
F16 = mybir.dt.float16
BF16 = mybir.dt.bfloat16
AF = mybir.ActivationFunctionType
OP = mybir.AluOpType


def build_program(c_scale: float):
    nc = bacc.Bacc("TRN2", debug=False)

    xt_d = nc.dram_tensor("xt", [D, N_PER], BF16, kind="ExternalInput").ap()
    xc_d = nc.dram_tensor("xc", [N_PER, D], F16, kind="ExternalInput").ap()
    ms_d = nc.dram_tensor("ms", [D, K], BF16, kind="ExternalInput").ap()
    lw_d = nc.dram_tensor("lw", [K, 1], F32, kind="ExternalInput").ap()
    nma_d = nc.dram_tensor("nma", [K, D + 1], BF16, kind="ExternalInput").ap()
    out_d = nc.dram_tensor("out", [N_PER, D], F16, kind="ExternalOutput").ap()

    inv_c = float(1.0 / c_scale)

    NP = NSB // 2  # pairs of super-blocks (DMA granularity)

    with tile.TileContext(nc) as tc, ExitStack() as ctx:
        consts = ctx.enter_context(tc.tile_pool(name="consts", bufs=1))
        xt_p = ctx.enter_context(tc.tile_pool(name="xt", bufs=3))
        xc_p = ctx.enter_context(tc.tile_pool(name="xc", bufs=3))
        eta_p = ctx.enter_context(tc.tile_pool(name="eta", bufs=4))
        small_p = ctx.enter_context(tc.tile_pool(name="small", bufs=4))
        out_p = ctx.enter_context(tc.tile_pool(name="outp", bufs=3))
        ps_st = ctx.enter_context(tc.tile_pool(name="ps_st", bufs=2, space="PSUM"))
        ps_v = ctx.enter_context(tc.tile_pool(name="ps_v", bufs=2, space="PSUM"))

        ms = consts.tile([D, K], BF16, name="ms")
        nc.sync.dma_start(ms, ms_d)
        lw = consts.tile([K, 1], F32, name="lw")
        nc.sync.dma_start(lw, lw_d)
        nma = consts.tile([K, D + 1], BF16, name="nma")
        nc.sync.dma_start(nma, nma_d)

        etas, xts, xcs, o2s = {}, {}, {}, {}

        def dma_in(p):
            n0 = p * 2 * SB
            # x^T slice (column-permuted: col 128g+q holds row 4q+g per SB)
            xt = xt_p.tile([128, 2 * SB], BF16, name="xt")
            nc.sync.dma_start(xt, xt_d[:, n0:n0 + 2 * SB])
            xts[p] = xt
            # c*x in f16, layout [q, (h g d)] <- rows n0 + 512h + 4q + g
            xc = xc_p.tile([128, 2 * SB], F16, name="xc")
            nc.gpsimd.dma_start(
                xc.rearrange("q (h g d) -> q h g d", g=4, d=D),
                xc_d[n0:n0 + 2 * SB, :].rearrange("(h q g) d -> q h g d", h=2, g=4),
            )
            xcs[p] = xc

        def head(s):
            # S^T[k, j] = x_j . m'_k / sigma2 ; E^T = exp(S^T + logw_adj)
            xt = xts[s // 2]
            h = s % 2
            pst = ps_st.tile([K, SB], F32, name="pst")
            nc.tensor.matmul(pst, lhsT=ms, rhs=xt[:, SB * h:SB * (h + 1)],
                             start=True, stop=True)
            eta = eta_p.tile([K, SB], BF16, name="eta")
            nc.scalar.activation(eta, pst, AF.Exp, bias=lw[:, 0:1], scale=1.0)
            etas[s] = eta

        def tail(s):
            eta = etas.pop(s)
            xc = xcs[s // 2]
            h = s % 2
            if h == 0:
                o2s[s // 2] = out_p.tile([128, 2 * SB], F16, name="o2")
            o2 = o2s[s // 2]
            # V_g = E_g @ [-c*m' | 1]; col 128 of each group = s/c
            pv01 = ps_v.tile([128, 2 * (D + 1)], F32, name="pv01")
            pv23 = ps_v.tile([128, 2 * (D + 1)], F32, name="pv23")
            for g in range(4):
                pv = pv01 if g < 2 else pv23
                lo = (g % 2) * (D + 1)
                nc.tensor.matmul(
                    pv[:, lo:lo + D + 1],
                    lhsT=eta[:, 128 * g:128 * (g + 1)],
                    rhs=nma, start=True, stop=True,
                )

            # out_g = V_g * (c/s) + c*x_g
            for j, pv in enumerate((pv01, pv23)):
                s_view = pv.rearrange("q (b c) -> q b c", c=D + 1)[:, :, D:D + 1]
                rc2 = small_p.tile([128, 2], F32, name="rc2")
                nc.vector.reciprocal(rc2, s_view)
                for jj in range(2):
                    g = 2 * j + jj
                    o_lo = SB * h + 128 * g
                    nc.vector.scalar_tensor_tensor(
                        out=o2[:, o_lo:o_lo + 128],
                        in0=pv[:, (D + 1) * jj:(D + 1) * jj + D],
                        scalar=rc2[:, jj:jj + 1],
                        in1=xc[:, o_lo:o_lo + 128],
                        op0=OP.mult,
                        op1=OP.add,
                    )

        def dma_out(p):
            n0 = p * 2 * SB
            xcs.pop(p)
            o2 = o2s.pop(p)
            nc.gpsimd.dma_start(
                out_d[n0:n0 + 2 * SB, :].rearrange("(h q g) d -> q h g d", h=2, g=4),
                o2.rearrange("q (h g d) -> q h g d", g=4, d=D),
            )

        for p in range(NP):
            dma_in(p)
            head(2 * p)
            if p:
                tail(2 * p - 2)
            head(2 * p + 1)
            if p:
                tail(2 * p - 1)
                dma_out(p - 1)
        tail(2 * NP - 2)
        tail(2 * NP - 1)
        dma_out(NP - 1)

    nc.compile()
    return nc


def _host_constants(means, weights, alphas_cumprod, t):
    acp = float(np.asarray(alphas_cumprod, dtype=np.float64)[int(t)])
    sigma2 = 1.0 - acp
    c = 1.0 / np.sqrt(sigma2)
    mprime = np.sqrt(acp) * np.asarray(means, dtype=np.float64)      # [K, D]

    ms = (mprime / sigma2).T.astype(np.float32)                      # [D, K]
    ms = ms.astype(ml_dtypes.bfloat16)

    # Scales folded into constants: E' = E/c (via -ln c in the bias) and
    # nma = [-c*m' | 1], so the ones column accumulates s/c and
    # out = (E'@nma) * (c/s) + c*x = -(E@m')*c/s + c*x directly.
    logw = np.log(np.asarray(weights, dtype=np.float64))
    lw = (logw - 0.5 * np.sum(mprime * mprime, axis=1) / sigma2 - np.log(c))
    lw = lw.astype(np.float32).reshape(K, 1).copy()

    nma = np.zeros((K, D + 1), dtype=np.float32)
    nma[:, :D] = (-c * mprime).astype(np.float32)
    nma[:, D] = 1.0
    nma = nma.astype(ml_dtypes.bfloat16)

    return float(c), ms, lw, nma


def _host_split_x(x, c):
    # Transposed bf16 copy, columns permuted so col (512s + 128g + p)
    # holds row (512s + 4p + g): mm2 stationary slices contiguous.
    v = x.reshape(-1, 128, 4, D)                        # [nsb_t, p, g, d]
    xt = np.ascontiguousarray(
        v.transpose(3, 0, 2, 1).reshape(D, -1).astype(ml_dtypes.bfloat16)
    )
    xc = (np.float32(c) * x).astype(np.float16)
    return xt, xc


def _build(inputs):
    x = np.ascontiguousarray(np.asarray(inputs["x"], dtype=np.float32))
    assert x.shape == (N, D), x.shape
    c, ms, lw, nma = _host_constants(
        inputs["means"], inputs["weights"], inputs["alphas_cumprod"], inputs["t"]
    )

    nc = build_program(c)
    in_maps = []
    for i in range(N_CORES):
        sl = slice(i * N_PER, (i + 1) * N_PER)
        xt, xc = _host_split_x(x[sl], c)
        in_maps.append({
            "xt": xt, "xc": xc,
            "ms": ms, "lw": lw, "nma": nma,
        })
    return nc, in_maps


def kernel(x, means, weights, alphas_cumprod, t):
    nc, in_maps = _build({
        "x": x, "means": means, "weights": weights,
        "alphas_cumprod": alphas_cumprod, "t": t,
    })
    res = run_bass_kernel_spmd(nc, in_maps, list(range(N_CORES)))
    out = np.concatenate([res.results[i]["out"] for i in range(N_CORES)], axis=0)
    return out.astype(np.float32, copy=False)


if __name__ == "__main__":
    rng = np.random.default_rng(0)
    x = rng.standard_normal((N, D), dtype=np.float32)
    means = 2.0 * rng.standard_normal((K, D)).astype(np.float32)
    w = rng.uniform(0.1, 1.0, K).astype(np.float32)
    weights = w / w.sum()
    betas = np.linspace(1e-4, 0.02, 1000, dtype=np.float32)
    acp = np.cumprod(1.0 - betas).astype(np.float32)
    out = kernel(x, means, weights, acp, 500)
    print("out", out.shape, out.dtype, out[:2, :4])


# revision 10
# speedup vs baseline: 2.0776x; 1.0423x over previous
"""Trainium2 Bass kernel for EpsilonNetGM (forward-diffused GMM score network).

Math (per row x of shape [D]):
    m'_k    = sqrt(acp) * means_k
    logit_k = (x . m'_k)/sigma2 + [log w_k - 0.5*||m'_k||^2/sigma2]
    resp    = softmax_k(logit)
    out     = c * (x - resp @ m'),   c = 1/sqrt(sigma2),  sigma2 = 1 - acp

Data-parallel over 8 NeuronCores: x/out sharded on the batch axis.

v3 — single-precision bf16 pipeline (tolerance is 2e-2; sim rel err 2.3e-3):
 - Host uploads x TWICE: transposed bf16 (for mm1's moving operand — no
   DMA/PE transposes on device) and c*x in f16 natural layout (for the
   final add). The transposed copy's columns are permuted so that
   n = 4p + g within each 512-row superblock: mm2 stationary slices stay
   contiguous AND the xc/out DMAs get 1KB-contiguous 4-row runs.
 - Per 512-row superblock: ONE 512-col mm1 (stationary ms [128,25]),
   one exp ACT (bias = logw_adj per-partition), FOUR 129-col mm2s
   (stationary = E^T 128-col slice, moving = [-m' | 1] so the softmax
   denominator lands in an extra PSUM column), four STTs
   out = V*(c/s) + c*x, f16 output store.
 - mm1 of superblock s+1 is issued before mm2 of superblock s so the PE
   never stalls on the exp latency.
"""

import os
import sys

for _p in ("/opt/trn_rl_repo", "/root/.axon_site/_ro/trn_rl_repo"):
    if os.path.isdir(_p) and _p not in sys.path:
        sys.path.insert(0, _p)

import numpy as np
import ml_dtypes
from contextlib import ExitStack

import concourse.bass as bass
import concourse.bacc as bacc
import concourse.tile as tile
from concourse import mybir
from concourse.bass_utils import run_bass_kernel_spmd

N_CORES = 8
N, K, D = 32768, 25, 128
N_PER = N // N_CORES          # 4096 rows per core
SB = 512                      # rows per super-block
NSB = N_PER // SB             # 8 super-blocks per core

F32 = mybir.dt.float32
F16 = mybir.dt.float16
BF16 = mybir.dt.bfloat16
AF = mybir.ActivationFunctionType
OP = mybir.AluOpType


def build_program(c_scale: float):
    nc = bacc.Bacc("TRN2", debug=False)

    xt_d = nc.dram_tensor("xt", [D, N_PER], BF16, kind="ExternalInput").ap()
    xc_d = nc.dram_tensor("xc", [N_PER, D], F16, kind="ExternalInput").ap()
    ms_d = nc.dram_tensor("ms", [D, K], BF16, kind="ExternalInput").ap()
    lw_d = nc.dram_tensor("lw", [K, 1], F32, kind="ExternalInput").ap()
    nma_d = nc.dram_tensor("nma", [K, D + 1], BF16, kind="ExternalInput").ap()
    out_d = nc.dram_tensor("out", [N_PER, D], F16, kind="ExternalOutput").ap()

    inv_c = float(1.0 / c_scale)

    NP = NSB // 2  # pairs of super-blocks (DMA granularity)

    with tile.TileContext(nc) as tc, ExitStack() as ctx:
        consts = ctx.enter_context(tc.tile_pool(name="consts", bufs=1))
        xt_p = ctx.enter_context(tc.tile_pool(name="xt", bufs=3))
        xc_p = ctx.enter_context(tc.tile_pool(name="xc", bufs=3))
        eta_p = ctx.enter_context(tc.tile_pool(name="eta", bufs=3))
        small_p = ctx.enter_context(tc.tile_pool(name="small", bufs=4))
        out_p = ctx.enter_context(tc.tile_pool(name="outp", bufs=3))
        ps_st = ctx.enter_context(tc.tile_pool(name="ps_st", bufs=2, space="PSUM"))
        ps_v = ctx.enter_context(tc.tile_pool(name="ps_v", bufs=2, space="PSUM"))

        etas, xts, xcs, o2s = {}, {}, {}, {}

        def dma_in(p, split=False):
            n0 = p * 2 * SB
            # x^T slice (column-permuted: col 128g+q holds row 4q+g per SB)
            xt = xt_p.tile([128, 2 * SB], BF16, name="xt")
            if split:  # pair 0: half-loads so mm1(0) starts sooner
                nc.sync.dma_start(xt[:, :SB], xt_d[:, n0:n0 + SB])
                nc.sync.dma_start(xt[:, SB:], xt_d[:, n0 + SB:n0 + 2 * SB])
            else:
                nc.sync.dma_start(xt, xt_d[:, n0:n0 + 2 * SB])
            xts[p] = xt
            # c*x in f16, layout [q, (h g d)] <- rows n0 + 512h + 4q + g
            xc = xc_p.tile([128, 2 * SB], F16, name="xc")
            nc.gpsimd.dma_start(
                xc.rearrange("q (h g d) -> q h g d", g=4, d=D),
                xc_d[n0:n0 + 2 * SB, :].rearrange("(h q g) d -> q h g d", h=2, g=4),
            )
            xcs[p] = xc

        def head2(p):
            # S^T[k, j] = x_j . m'_k / sigma2 for both SBs of the pair,
            # then ONE exp ACT over [K, 1024].
            xt = xts[p]
            pst = ps_st.tile([K, 2 * SB], F32, name="pst")
            nc.tensor.matmul(pst[:, :SB], lhsT=ms, rhs=xt[:, :SB],
                             start=True, stop=True)
            nc.tensor.matmul(pst[:, SB:], lhsT=ms, rhs=xt[:, SB:],
                             start=True, stop=True)
            eta = eta_p.tile([K, 2 * SB], BF16, name="eta")
            nc.scalar.activation(eta, pst, AF.Exp, bias=lw[:, 0:1], scale=1.0)
            etas[p] = eta

        def tail(s):
            p, h = divmod(s, 2)
            eta = etas[p]
            xc = xcs[p]
            if h == 0:
                o2s[p] = out_p.tile([128, 2 * SB], F16, name="o2")
            o2 = o2s[p]
            # V_g = E_g @ [-c*m' | 1]; col 128 of each 129-group = s/c.
            # One 2-bank PSUM tile [q, i, 512]: group g at (i=g//2, 129*(g%2)).
            pv = ps_v.tile([128, 2, SB], F32, name="pv")
            for g in range(4):
                i, j = divmod(g, 2)
                nc.tensor.matmul(
                    pv[:, i:i + 1, (D + 1) * j:(D + 1) * j + D + 1],
                    lhsT=eta[:, SB * h + 128 * g:SB * h + 128 * (g + 1)],
                    rhs=nma, start=True, stop=True,
                )

            # rc = c/s for all 4 groups in one reciprocal
            rc4 = small_p.tile([128, 4], F32, name="rc4")
            nc.vector.reciprocal(
                rc4.rearrange("q (i j w) -> q i j w", j=2, w=1),
                pv[:, :, :2 * (D + 1)].rearrange(
                    "q i (j y) -> q i j y", y=D + 1)[:, :, :, D:D + 1],
            )
            # out_g = V_g * (c/s) + c*x_g
            for g in range(4):
                i, j = divmod(g, 2)
                o_lo = SB * h + 128 * g
                nc.vector.scalar_tensor_tensor(
                    out=o2[:, o_lo:o_lo + 128],
                    in0=pv[:, i:i + 1, (D + 1) * j:(D + 1) * j + D],
                    scalar=rc4[:, g:g + 1],
                    in1=xc[:, o_lo:o_lo + 128],
                    op0=OP.mult,
                    op1=OP.add,
                )

        def dma_out(p, eng=None, split=False):
            n0 = p * 2 * SB
            o2 = o2s.pop(p)
            src = o2.rearrange("q (h g d) -> q h g d", g=4, d=D)
            dst = out_d[n0:n0 + 2 * SB, :].rearrange(
                "(h q g) d -> q h g d", h=2, g=4)
            if split:  # last pair: halves on separate queues, right away
                nc.sync.dma_start(dst[:, 0:1], src[:, 0:1])
                nc.gpsimd.dma_start(dst[:, 1:2], src[:, 1:2])
            else:
                eng.dma_start(dst, src)

        # consts go on the Scalar queue (idle at start) so the Sync queue
        # issues the first xt load immediately.
        ms = consts.tile([D, K], BF16, name="ms")
        nc.scalar.dma_start(ms, ms_d)
        lw = consts.tile([K, 1], F32, name="lw")
        nc.scalar.dma_start(lw, lw_d)
        nma = consts.tile([K, D + 1], BF16, name="nma")
        nc.scalar.dma_start(nma, nma_d)

        for p in range(NP):
            dma_in(p, split=(p == 0))
            head2(p)
            if p:
                tail(2 * p - 1)
                dma_out(p - 1, eng=nc.sync if p % 2 else nc.gpsimd)
            tail(2 * p)
            if p == NP - 1:
                tail(2 * p + 1)
                dma_out(p, split=True)

    nc.compile()
    return nc


def _host_constants(means, weights, alphas_cumprod, t):
    acp = float(np.asarray(alphas_cumprod, dtype=np.float64)[int(t)])
    sigma2 = 1.0 - acp
    c = 1.0 / np.sqrt(sigma2)
    mprime = np.sqrt(acp) * np.asarray(means, dtype=np.float64)      # [K, D]

    ms = (mprime / sigma2).T.astype(np.float32)                      # [D, K]
    ms = ms.astype(ml_dtypes.bfloat16)

    # Scales folded into constants: E' = E/c (via -ln c in the bias) and
    # nma = [-c*m' | 1], so the ones column accumulates s/c and
    # out = (E'@nma) * (c/s) + c*x = -(E@m')*c/s + c*x directly.
    logw = np.log(np.asarray(weights, dtype=np.float64))
    lw = (logw - 0.5 * np.sum(mprime * mprime, axis=1) / sigma2 - np.log(c))
    lw = lw.astype(np.float32).reshape(K, 1).copy()

    nma = np.zeros((K, D + 1), dtype=np.float32)
    nma[:, :D] = (-c * mprime).astype(np.float32)
    nma[:, D] = 1.0
    nma = nma.astype(ml_dtypes.bfloat16)

    return float(c), ms, lw, nma


def _host_split_x(x, c):
    # Transposed bf16 copy, columns permuted so col (512s + 128g + p)
    # holds row (512s + 4p + g): mm2 stationary slices contiguous.
    v = x.reshape(-1, 128, 4, D)                        # [nsb_t, p, g, d]
    xt = np.ascontiguousarray(
        v.transpose(3, 0, 2, 1).reshape(D, -1).astype(ml_dtypes.bfloat16)
    )
    xc = (np.float32(c) * x).astype(np.float16)
    return xt, xc


def _build(inputs):
    x = np.ascontiguousarray(np.asarray(inputs["x"], dtype=np.float32))
    assert x.shape == (N, D), x.shape
    c, ms, lw, nma = _host_constants(
        inputs["means"], inputs["weights"], inputs["alphas_cumprod"], inputs["t"]
    )

    nc = build_program(c)
    in_maps = []
    for i in range(N_CORES):
        sl = slice(i * N_PER, (i + 1) * N_PER)
        xt, xc = _host_split_x(x[sl], c)
        in_maps.append({
            "xt": xt, "xc": xc,
            "ms": ms, "lw": lw, "nma": nma,
        })
    return nc, in_maps


def kernel(x, means, weights, alphas_cumprod, t):
    nc, in_maps = _build({
        "x": x, "means": means, "weights": weights,
        "alphas_cumprod": alphas_cumprod, "t": t,
    })
    res = run_bass_kernel_spmd(nc, in_maps, list(range(N_CORES)))
    out = np.concatenate([res.results[i]["out"] for i in range(N_CORES)], axis=0)
    return out.astype(np.float32, copy=False)


if __name__ == "__main__":
    rng = np.random.default_rng(0)
    x = rng.standard_normal((N, D), dtype=np.float32)
    means = 2.0 * rng.standard_normal((K, D)).astype(np.float32)
    w = rng.uniform(0.1, 1.0, K).astype(np.float32)
    weights = w / w.sum()
    betas = np.linspace(1e-4, 0.02, 1000, dtype=np.float32)
    acp = np.cumprod(1.0 - betas).astype(np.float32)
    out = kernel(x, means, weights, acp, 500)
    print("out", out.shape, out.dtype, out[:2, :4])


# revision 14
# speedup vs baseline: 2.1319x; 1.0261x over previous
"""Trainium2 Bass kernel for EpsilonNetGM (forward-diffused GMM score network).

Math (per row x of shape [D]):
    m'_k    = sqrt(acp) * means_k
    logit_k = (x . m'_k)/sigma2 + [log w_k - 0.5*||m'_k||^2/sigma2]
    resp    = softmax_k(logit)
    out     = c * (x - resp @ m'),   c = 1/sqrt(sigma2),  sigma2 = 1 - acp

Data-parallel over 8 NeuronCores: x/out sharded on the batch axis.

v3 — single-precision bf16 pipeline (tolerance is 2e-2; sim rel err 2.3e-3):
 - Host uploads x TWICE: transposed bf16 (for mm1's moving operand — no
   DMA/PE transposes on device) and c*x in f16 natural layout (for the
   final add). The transposed copy's columns are permuted so that
   n = 4p + g within each 512-row superblock: mm2 stationary slices stay
   contiguous AND the xc/out DMAs get 1KB-contiguous 4-row runs.
 - Per 512-row superblock: ONE 512-col mm1 (stationary ms [128,25]),
   one exp ACT (bias = logw_adj per-partition), FOUR 129-col mm2s
   (stationary = E^T 128-col slice, moving = [-m' | 1] so the softmax
   denominator lands in an extra PSUM column), four STTs
   out = V*(c/s) + c*x, f16 output store.
 - mm1 of superblock s+1 is issued before mm2 of superblock s so the PE
   never stalls on the exp latency.
"""

import os
import sys

for _p in ("/opt/trn_rl_repo", "/root/.axon_site/_ro/trn_rl_repo"):
    if os.path.isdir(_p) and _p not in sys.path:
        sys.path.insert(0, _p)

import numpy as np
import ml_dtypes
from contextlib import ExitStack

import concourse.bass as bass
import concourse.bacc as bacc
import concourse.tile as tile
from concourse import mybir
from concourse.bass_utils import run_bass_kernel_spmd

N_CORES = 8
N, K, D = 32768, 25, 128
N_PER = N // N_CORES          # 4096 rows per core
SB = 512                      # rows per super-block
NSB = N_PER // SB             # 8 super-blocks per core

F32 = mybir.dt.float32
F16 = mybir.dt.float16
BF16 = mybir.dt.bfloat16
AF = mybir.ActivationFunctionType
OP = mybir.AluOpType


def build_program(c_scale: float):
    nc = bacc.Bacc("TRN2", debug=False)

    xt_d = nc.dram_tensor("xt", [D, N_PER], BF16, kind="ExternalInput").ap()
    xc_d = nc.dram_tensor("xc", [N_PER, D], F16, kind="ExternalInput").ap()
    ms_d = nc.dram_tensor("ms", [D, K], BF16, kind="ExternalInput").ap()
    lw_d = nc.dram_tensor("lw", [K, 1], F32, kind="ExternalInput").ap()
    nma_d = nc.dram_tensor("nma", [K, D + 1], BF16, kind="ExternalInput").ap()
    out_d = nc.dram_tensor("out", [N_PER, D], F16, kind="ExternalOutput").ap()

    inv_c = float(1.0 / c_scale)

    NP = NSB // 2  # pairs of super-blocks (DMA granularity)

    with tile.TileContext(nc) as tc, ExitStack() as ctx:
        consts = ctx.enter_context(tc.tile_pool(name="consts", bufs=1))
        xt_p = ctx.enter_context(tc.tile_pool(name="xt", bufs=3))
        xc_p = ctx.enter_context(tc.tile_pool(name="xc", bufs=3))
        eta_p = ctx.enter_context(tc.tile_pool(name="eta", bufs=4))
        small_p = ctx.enter_context(tc.tile_pool(name="small", bufs=4))
        out_p = ctx.enter_context(tc.tile_pool(name="outp", bufs=3))
        ps_st = ctx.enter_context(tc.tile_pool(name="ps_st", bufs=2, space="PSUM"))
        ps_v = ctx.enter_context(tc.tile_pool(name="ps_v", bufs=3, space="PSUM"))

        etas, xts, xcs, o2s = {}, {}, {}, {}

        def dma_in(p, split=False):
            n0 = p * 2 * SB
            # x^T slice (column-permuted: col 128g+q holds row 4q+g per SB)
            xt = xt_p.tile([128, 2 * SB], BF16, name="xt")
            if split:  # pair 0: half-loads so mm1(0) starts sooner
                nc.sync.dma_start(xt[:, :SB], xt_d[:, n0:n0 + SB])
                nc.sync.dma_start(xt[:, SB:], xt_d[:, n0 + SB:n0 + 2 * SB])
            else:
                # alternate queues -> two concurrent input streams
                eng = nc.scalar if p % 2 else nc.sync
                eng.dma_start(xt, xt_d[:, n0:n0 + 2 * SB])
            xts[p] = xt
            # c*x in f16, layout [q, (h g d)] <- rows n0 + 512h + 4q + g
            xc = xc_p.tile([128, 2 * SB], F16, name="xc")
            nc.gpsimd.dma_start(
                xc.rearrange("q (h g d) -> q h g d", g=4, d=D),
                xc_d[n0:n0 + 2 * SB, :].rearrange("(h q g) d -> q h g d", h=2, g=4),
            )
            xcs[p] = xc

        def head(s):
            # S^T[k, j] = x_j . m'_k / sigma2 ; E^T = exp(S^T + logw_adj)
            p, h = divmod(s, 2)
            xt = xts[p]
            pst = ps_st.tile([K, SB], F32, name="pst")
            nc.tensor.matmul(pst, lhsT=ms, rhs=xt[:, SB * h:SB * (h + 1)],
                             start=True, stop=True)
            eta = eta_p.tile([K, SB], BF16, name="eta")
            nc.scalar.activation(eta, pst, AF.Exp, bias=lw[:, 0:1], scale=1.0)
            etas[s] = eta

        def tail(s):
            p, h = divmod(s, 2)
            eta = etas.pop(s)
            xc = xcs[p]
            if h == 0:
                o2s[p] = out_p.tile([128, 2 * SB], F16, name="o2")
            o2 = o2s[p]
            # V_g = E_g @ [-c*m' | 1]; col 128 of each 129-group = s/c.
            # One 2-bank PSUM tile [q, i, 512]: group g at (i=g//2, 129*(g%2)).
            pv = ps_v.tile([128, 2, SB], F32, name="pv")
            for g in range(4):
                i, j = divmod(g, 2)
                nc.tensor.matmul(
                    pv[:, i:i + 1, (D + 1) * j:(D + 1) * j + D + 1],
                    lhsT=eta[:, 128 * g:128 * (g + 1)],
                    rhs=nma, start=True, stop=True,
                )

            # rc = c/s for all 4 groups in one reciprocal
            rc4 = small_p.tile([128, 4], F32, name="rc4")
            nc.vector.reciprocal(
                rc4.rearrange("q (i j w) -> q i j w", j=2, w=1),
                pv[:, :, :2 * (D + 1)].rearrange(
                    "q i (j y) -> q i j y", y=D + 1)[:, :, :, D:D + 1],
            )
            # out_g = V_g * (c/s) + c*x_g
            for g in range(4):
                i, j = divmod(g, 2)
                o_lo = SB * h + 128 * g
                nc.vector.scalar_tensor_tensor(
                    out=o2[:, o_lo:o_lo + 128],
                    in0=pv[:, i:i + 1, (D + 1) * j:(D + 1) * j + D],
                    scalar=rc4[:, g:g + 1],
                    in1=xc[:, o_lo:o_lo + 128],
                    op0=OP.mult,
                    op1=OP.add,
                )

        def dma_out(p, eng=None, split=False):
            n0 = p * 2 * SB
            o2 = o2s.pop(p)
            src = o2.rearrange("q (h g d) -> q h g d", g=4, d=D)
            dst = out_d[n0:n0 + 2 * SB, :].rearrange(
                "(h q g) d -> q h g d", h=2, g=4)
            if split:  # last pair: halves on separate queues, right away
                nc.sync.dma_start(dst[:, 0:1], src[:, 0:1])
                nc.gpsimd.dma_start(dst[:, 1:2], src[:, 1:2])
            else:
                eng.dma_start(dst, src)

        # consts go on the Scalar queue (idle at start) so the Sync queue
        # issues the first xt load immediately.
        ms = consts.tile([D, K], BF16, name="ms")
        nc.scalar.dma_start(ms, ms_d)
        lw = consts.tile([K, 1], F32, name="lw")
        nc.scalar.dma_start(lw, lw_d)
        nma = consts.tile([K, D + 1], BF16, name="nma")
        nc.scalar.dma_start(nma, nma_d)

        for p in range(NP):
            dma_in(p, split=(p == 0))
            head(2 * p)
            head(2 * p + 1)
            if p:
                tail(2 * p - 1)
                dma_out(p - 1, eng=nc.sync if p % 2 else nc.gpsimd)
            tail(2 * p)
            if p == NP - 1:
                tail(2 * p + 1)
                dma_out(p, split=True)

    nc.compile()
    return nc


def _host_constants(means, weights, alphas_cumprod, t):
    acp = float(np.asarray(alphas_cumprod, dtype=np.float64)[int(t)])
    sigma2 = 1.0 - acp
    c = 1.0 / np.sqrt(sigma2)
    mprime = np.sqrt(acp) * np.asarray(means, dtype=np.float64)      # [K, D]

    ms = (mprime / sigma2).T.astype(np.float32)                      # [D, K]
    ms = ms.astype(ml_dtypes.bfloat16)

    # Scales folded into constants: E' = E/c (via -ln c in the bias) and
    # nma = [-c*m' | 1], so the ones column accumulates s/c and
    # out = (E'@nma) * (c/s) + c*x = -(E@m')*c/s + c*x directly.
    logw = np.log(np.asarray(weights, dtype=np.float64))
    lw = (logw - 0.5 * np.sum(mprime * mprime, axis=1) / sigma2 - np.log(c))
    lw = lw.astype(np.float32).reshape(K, 1).copy()

    nma = np.zeros((K, D + 1), dtype=np.float32)
    nma[:, :D] = (-c * mprime).astype(np.float32)
    nma[:, D] = 1.0
    nma = nma.astype(ml_dtypes.bfloat16)

    return float(c), ms, lw, nma


def _host_split_x(x, c):
    # Transposed bf16 copy, columns permuted so col (512s + 128g + p)
    # holds row (512s + 4p + g): mm2 stationary slices contiguous.
    v = x.reshape(-1, 128, 4, D)                        # [nsb_t, p, g, d]
    xt = np.ascontiguousarray(
        v.transpose(3, 0, 2, 1).reshape(D, -1).astype(ml_dtypes.bfloat16)
    )
    xc = (np.float32(c) * x).astype(np.float16)
    return xt, xc


def _build(inputs):
    x = np.ascontiguousarray(np.asarray(inputs["x"], dtype=np.float32))
    assert x.shape == (N, D), x.shape
    c, ms, lw, nma = _host_constants(
        inputs["means"], inputs["weights"], inputs["alphas_cumprod"], inputs["t"]
    )

    nc = build_program(c)
    in_maps = []
    for i in range(N_CORES):
        sl = slice(i * N_PER, (i + 1) * N_PER)
        xt, xc = _host_split_x(x[sl], c)
        in_maps.append({
            "xt": xt, "xc": xc,
            "ms": ms, "lw": lw, "nma": nma,
        })
    return nc, in_maps


def kernel(x, means, weights, alphas_cumprod, t):
    nc, in_maps = _build({
        "x": x, "means": means, "weights": weights,
        "alphas_cumprod": alphas_cumprod, "t": t,
    })
    res = run_bass_kernel_spmd(nc, in_maps, list(range(N_CORES)))
    out = np.concatenate([res.results[i]["out"] for i in range(N_CORES)], axis=0)
    return out.astype(np.float32, copy=False)


if __name__ == "__main__":
    rng = np.random.default_rng(0)
    x = rng.standard_normal((N, D), dtype=np.float32)
    means = 2.0 * rng.standard_normal((K, D)).astype(np.float32)
    w = rng.uniform(0.1, 1.0, K).astype(np.float32)
    weights = w / w.sum()
    betas = np.linspace(1e-4, 0.02, 1000, dtype=np.float32)
    acp = np.cumprod(1.0 - betas).astype(np.float32)
    out = kernel(x, means, weights, acp, 500)
    print("out", out.shape, out.dtype, out[:2, :4])
